# revision 2
# baseline (speedup 1.0000x reference)
"""Trainium2 Bass kernel v3 for nn_Attention_16612933501279.

Same algebra as v1 (see kernel.py docstring), restructured schedule:
  - c DRAM layout [128, B, H2, KC, NH, PIX]: 8 half-batch chunk DMAs,
    8KB/partition contiguous both sides, chained depth-3.
  - per-batch stats PSUM tiles (bufs=2): ssq h0/h1 at strips 0/1,
    dots h0/h1 at strips 2/3, rows j=0..3; stats matmuls interleaved
    across all 4 column strips.
  - per-batch softmax chain on 36-partition tiles: ACT Rsqrt / Exp /
    Reciprocal tables + 3 DVE muls; dots read from PSUM base 64/96.
  - w broadcast via DRAM bounce for every batch (no engine time).
  - squares kc0 on DVE, kc1 on ACT/GpSimd; 4 prod tiles on GpSimd.
"""

import sys

import numpy as np

try:
    import concourse.bass as bass  # noqa: F401
except ImportError:
    sys.path.insert(0, "/opt/trn_rl_repo")

import concourse.bass as bass
import concourse.mybir as mybir
from concourse import bacc, tile
from concourse import dve_ops as _dve_ops
from concourse.bass_utils import run_bass_kernel_spmd
from concourse.dve_ops import DveOp
from concourse.dve_spec import C0, C1, C2, C3, Spec, Src0, Src1, lower, sq
from concourse.dve_spec import _has_src1 as has_src1
from concourse.dve_spec import _spill_c3_to_src1
from concourse.dve_uop import DveOpSpec

AF = mybir.ActivationFunctionType
ALU = mybir.AluOpType
BF16 = mybir.dt.bfloat16
F32 = mybir.dt.float32

B, N, C, H, W = 4, 8, 256, 64, 64
D = 512
NCORES = 8
HS = H // NCORES
PIX = HS * W              # 512 pixels per (b, n) tile per core
KC = C // 128
H2 = 2                    # token halves
NH = N // H2              # tokens per half (4)

N_WARM = 16

SQRT_C = float(np.sqrt(C))
# fused quadratic-seed + Newton rsqrt constants (fit on ssq in [143,585])
QFA = 1.557625210367e-07
QFB = -1.840504126191e-04
QFD = 8.774860444241e-02
QFC = 1.889881574842

# const tile free-axis layout (bf16 elements); solo stats rows j
SSQS_OFF = 0                        # j -> [128,4] one-hot col j (ones)
DOTSS_OFF = SSQS_OFF + 4 * 4        # (b,kc,j) -> [128,4] col j = wq2
ZSELS_OFF = DOTSS_OFF + B * KC * NH * 4  # [36,4]: ones rows 0-3,32-35
ID_OFF = ZSELS_OFF + 4              # [128,128] identity
MWT_OFF = ID_OFF + 128              # (kc,mc) -> [128,128] Mw2 chunk.T
BSEL_OFF = MWT_OFF + KC * C         # j -> [?,128] ones-row j (both strips)
CONST_W = BSEL_OFF + NH * 128

# squares engine per (b, h, kc): v=DVE, a=ACT.  GpSimd is banned: any
# DVE op overlapping a GpSimd op runs 3-4x slower (SBUF contention).
SQ_ENG = {
    (0, 0, 0): "v", (0, 0, 1): "a", (0, 1, 0): "v", (0, 1, 1): "a",
    (1, 0, 0): "v", (1, 0, 1): "a", (1, 1, 0): "v", (1, 1, 1): "v",
    (2, 0, 0): "v", (2, 0, 1): "a", (2, 1, 0): "v", (2, 1, 1): "a",
    (3, 0, 0): "v", (3, 0, 1): "a", (3, 1, 0): "v", (3, 1, 1): "a",
}


def _register_op(name, spec_body, spec_ref):
    for op in _dve_ops.OPS:
        if op.name == name:
            return op
    spec = Spec(body=spec_body, reference=spec_ref)
    sub = _dve_ops._CUSTOM_DVE_ROW_BASE + len(_dve_ops.OPS)
    assert sub < 0x20
    shas = {}
    for ver in ("v3", "v4"):
        try:
            s = DveOpSpec(name=name, opcode=sub, uops=lower(spec, ver=ver),
                          rd1_en=has_src1(spec))
            shas[ver] = s.sha(ver)
        except Exception:
            pass
    op = DveOp(name, spec, subdim=False, uops_sha=shas)
    _dve_ops.OPS.append(op)
    _dve_ops._SUB_OPCODE_FOR_NAME[name] = sub
    _dve_ops.CUSTOM_DVE_SPECS[name] = spec
    return op


_Y = (Src0 * C0 + C1) * Src0 + C2
RSQRT_F = _register_op(
    "ANT_RSQRT_F1_ATT",
    _spill_c3_to_src1(_Y * (C3 - Src0 * sq(_Y))),
    lambda in0, in1, c0, c1, c2: ((in0 * c0 + c1) * in0 + c2)
    * (in1 - in0 * ((in0 * c0 + c1) * in0 + c2) ** 2),
)


def _build_nc():
    nc = bacc.Bacc(None, target_bir_lowering=False)
    c_d = nc.declare_dram_parameter(
        "c", [128, B, H2, KC, NH, PIX], BF16, isOutput=False)
    k_d = nc.declare_dram_parameter("consts", [128, CONST_W], BF16,
                                    isOutput=False)
    bo_d = nc.declare_dram_parameter("bo2", [128, KC + 1], F32,
                                     isOutput=False)
    out_d = nc.declare_dram_parameter("out", [B, C, HS, W], BF16,
                                      isOutput=True)
    w_dram = nc.dram_tensor("w_scratch", [B, H2, NH, PIX], BF16,
                            kind="Internal")

    with (
        tile.TileContext(nc) as tc,
        tc.tile_pool(name="const", bufs=1) as cpool,
        tc.tile_pool(name="work", bufs=4) as work,
        tc.tile_pool(name="small", bufs=3) as small,
        tc.tile_pool(name="psum", bufs=1, space="PSUM") as pp,
    ):
        consts = cpool.tile([128, CONST_W], BF16, tag="consts")
        bo_sb = cpool.tile([128, KC + 1], F32, tag="bo")
        c_sb = [cpool.tile([128, H2, KC, NH, PIX], BF16, tag=f"c{b}",
                           name=f"c{b}") for b in range(B)]
        nc.sync.dma_start(consts[:], k_d[:])
        nc.sync.dma_start(bo_sb[:], bo_d[:])
        # force the ACT table load before any real ACT op needs it
        actwarm = cpool.tile([128, 16], BF16, tag="actwarm")
        nc.scalar.activation(actwarm[:], consts[:, 0:16], AF.Square)
        cdmas = []
        for b in range(B):
            for h in range(H2):
                ins = nc.sync.dma_start(c_sb[b][:, h], c_d[:, b, h])
                if len(cdmas) >= 3:
                    tile.add_dep_helper(
                        ins.ins, cdmas[-3].ins,
                        reason="pipeline input DMAs depth-3",
                    )
                cdmas.append(ins)

        def st_ssq(b, bip, j):
            o = SSQS_OFF + j * 4
            return consts[:, o : o + 4]

        def st_dots(b, bip, kc, j):
            o = DOTSS_OFF + ((b * KC + kc) * NH + j) * 4
            return consts[:, o : o + 4]

        zsel_s = consts[0:36, ZSELS_OFF : ZSELS_OFF + 4]
        ident = consts[:, ID_OFF : ID_OFF + 128]

        def st_mwt(kc, mc):
            o = MWT_OFF + kc * C + mc * 128
            return consts[:, o : o + 128]

        def st_bsel(h, j):
            o = BSEL_OFF + j * 128
            return consts[32 * h : 32 * h + 4, o : o + 128]

        # PSUM: stats 2 + z 1 + cm 2 + ops 2 = 7 banks
        stats = {}

        zwarm = pp.tile([4, PIX], F32, tag="z", bufs=1, name="zwarm")
        for _ in range(N_WARM):
            nc.tensor.matmul(zwarm[:], st_ssq(0, 0, 0), consts[:, 0:PIX],
                             start=True, stop=True)

        sq_done = {}

        def emit_squares(b, h):
            for kc in range(KC):
                csq = work.tile([128, NH, PIX], BF16, tag="csq", bufs=4,
                                name="csq")
                src_ = c_sb[b][:, h, kc]
                eng = SQ_ENG[(b, h, kc)]
                if eng == "v":
                    nc.vector.tensor_mul(csq[:], src_, src_)
                else:
                    # halves: caps ACT-queue blocking of chain exps
                    nc.scalar.activation(csq[:, 0:2, :], src_[:, 0:2, :],
                                         AF.Square)
                    nc.scalar.activation(csq[:, 2:4, :], src_[:, 2:4, :],
                                         AF.Square)
                sq_done[(b, h, kc)] = csq

        def emit_stats(b, h_outer=False):
            """Stats for batch b across 4 col strips.

            b0/b1 share a pair tile (rows bip*4+j); b2/b3 get solo
            tiles (rows j).  h_outer emits all h0 matmuls before h1
            (keeps PE busy while h1 data is still in flight)."""
            bip = 0
            stats[b] = stp = pp.tile([128, PIX], F32, tag="stats",
                                     bufs=2, name=f"stats{b}")
            nrow = 4
            first0 = last1 = True
            orders = ([(h, kc, j) for h in range(H2) for kc in range(KC)
                       for j in range(NH)] if h_outer else
                      [(h, kc, j) for kc in range(KC) for j in range(NH)
                       for h in range(H2)])
            for h, kc, j in orders:
                csq = sq_done[(b, h, kc)]
                first = first0 and kc == 0 and j == 0
                last = last1 and kc == KC - 1 and j == NH - 1
                sbase = 32 * h
                dbase = 64 + 32 * h
                nc.tensor.matmul(
                    stp[sbase : sbase + nrow, :], st_ssq(b, bip, j),
                    csq[:, j, :],
                    start=first, stop=last,
                    tile_position=(0, sbase),
                )
                nc.tensor.matmul(
                    stp[dbase : dbase + nrow, :], st_dots(b, bip, kc, j),
                    c_sb[b][:, h, kc, j, :],
                    start=first, stop=last,
                    tile_position=(0, dbase),
                )

        def emit_chain(b):
            with tc.high_priority(offset=60):
                return _emit_chain(b)

        def _emit_chain(b):
            """Solo softmax chain, 36 rows.  w = e * (s * zinv)."""
            nr = 36
            stp = stats[b]
            ssq = stp[0:nr, :]
            dots = stp[64 : 64 + nr, :]
            zsel = zsel_s
            zr = 4
            s_sb = small.tile([nr, PIX], BF16, tag="s", bufs=2)
            nc.vector._custom_dve(
                RSQRT_F, out=s_sb[:], in0=ssq,
                in1=bo_sb[0:nr, KC : KC + 1],
                s0=QFA, s1=QFB, imm2=QFD,
            )
            dscl = small.tile([nr, PIX], F32, tag="dscl")
            nc.vector.tensor_mul(dscl[:], dots, s_sb[:])
            e_sb = small.tile([nr, PIX], BF16, tag="e", bufs=2)
            nc.scalar.activation(e_sb[:], dscl[:], AF.Exp)
            zpt = pp.tile([128, PIX], F32, tag="z", bufs=1, name="z")
            nc.tensor.matmul(zpt[0:zr, :], zsel, e_sb[:], start=True,
                             stop=True, tile_position=(0, 0))
            nc.tensor.matmul(zpt[32 : 32 + zr, :], zsel, e_sb[:],
                             start=True, stop=True, tile_position=(0, 32))
            zinv = small.tile([nr, PIX], F32, tag="zinv")
            nc.vector.reciprocal_approx_fast(zinv[:], zpt[0:nr, :])
            szi = small.tile([nr, PIX], BF16, tag="szi")
            nc.vector.tensor_mul(szi[:], s_sb[:], zinv[:])
            w_sb = small.tile([nr, PIX], BF16, tag="w", bufs=2)
            nc.vector.tensor_mul(w_sb[:], e_sb[:], szi[:])
            return w_sb

        def emit_pe_bcast(b, w_sb):
            """PE ones-matmul broadcast + ACT copies (no DRAM hop)."""
            wbt = work.tile([128, H2, NH, PIX], BF16, tag="wbt", bufs=4,
                            name="wbt")
            for h in range(H2):
                for jp in range(0, NH, 2):
                    wps = pp.tile([128, 2, PIX], F32, tag="stats",
                                  bufs=2, name="wps")
                    for dj in range(2):
                        nc.tensor.matmul(
                            wps[:, dj, :], st_bsel(h, jp + dj),
                            w_sb[32 * h : 32 * h + 4, :],
                            start=True, stop=True,
                        )
                    nc.scalar.copy(wbt[:, h, jp : jp + 2, :], wps[:])
            return wbt

        def emit_bounce(b, w_sb):
            """Bounce w to DRAM, broadcast back across 128 partitions."""
            bip = 0
            wbt = work.tile([128, H2, NH, PIX], BF16, tag="wbt", bufs=4,
                            name="wbt")
            for h in range(H2):
                nc.sync.dma_start(
                    w_dram[b, h],
                    w_sb[32 * h + 4 * bip : 32 * h + 4 * bip + 4, :])
            for h in range(H2):
                nc.sync.dma_start(
                    wbt[:, h],
                    w_dram[b, h][None].to_broadcast((128, NH, PIX)),
                )
            return wbt

        prod_store = {}

        def emit_prods(b, wbt):
            if b not in prod_store:
                prod_store[b] = work.tile(
                    [128, H2, KC, NH, PIX], BF16, tag="prod", bufs=2,
                    name="prod")
            prod = prod_store[b]
            for kc in range(KC):
                for h in range(H2):
                    nc.vector.tensor_mul(
                        prod[:, h, kc], c_sb[b][:, h, kc], wbt[:, h])

        def emit_accum_out(b, cmix_eng="a"):
            prod = prod_store[b]
            cm = pp.tile([128, KC, PIX], F32, tag="cm", bufs=1, name="cm")
            for kc in range(KC):
                for h in range(H2):
                    for j in range(NH):
                        nc.tensor.matmul(
                            cm[:, kc, :], ident, prod[:, h, kc, j, :],
                            start=(h == 0 and j == 0),
                            stop=(h == 1 and j == NH - 1),
                        )
            cmix = work.tile([128, KC, PIX], BF16, tag="cmix", bufs=2,
                             name="cmix")
            if cmix_eng == "a":
                nc.scalar.copy(cmix[:], cm[:])
            else:
                nc.vector.tensor_copy(cmix[:], cm[:])
            osb = work.tile([128, KC, PIX], BF16, tag="osb", bufs=2,
                            name="osb")
            for mc in range(KC):
                ops = pp.tile([128, PIX], F32, tag="ops", bufs=2,
                              name="ops")
                for kc in range(KC):
                    nc.tensor.matmul(
                        ops[:], st_mwt(kc, mc), cmix[:, kc, :],
                        start=(kc == 0), stop=(kc == KC - 1),
                    )
                nc.scalar.activation(
                    osb[:, mc, :], ops[:], AF.Identity,
                    bias=bo_sb[:, mc : mc + 1], scale=1.0,
                )
                nc.scalar.dma_start(
                    out_d[b].rearrange("(mc p) h w -> mc p (h w)", p=128)[mc],
                    osb[:, mc, :],
                )

        # ---- emission order (software pipeline, priority hints) ----
        wbts = {}
        emit_squares(0, 0)
        emit_squares(0, 1)
        emit_stats(0, h_outer=True)
        w0 = emit_chain(0)
        wbts[0] = emit_bounce(0, w0)
        emit_squares(1, 0)
        emit_squares(1, 1)
        emit_stats(1)
        w1 = emit_chain(1)
        wbts[1] = emit_bounce(1, w1)
        emit_squares(2, 0)
        emit_squares(2, 1)
        emit_stats(2)
        emit_prods(0, wbts[0])
        w2 = emit_chain(2)
        wbts[2] = emit_bounce(2, w2)
        emit_squares(3, 0)
        emit_squares(3, 1)
        emit_stats(3)
        emit_prods(1, wbts[1])
        w3 = emit_chain(3)
        wbts[3] = emit_bounce(3, w3)
        emit_accum_out(0)
        emit_prods(2, wbts[2])
        emit_accum_out(1)
        emit_prods(3, wbts[3])
        emit_accum_out(2)
        emit_accum_out(3)

    nc.compile()
    return nc


def _host_consts(q, g, Wq, Wkv, Wo, bo):
    bf = mybir.dt.np(BF16)
    q, g, Wq, Wkv, Wo, bo = (
        np.asarray(x, np.float32) for x in (q, g, Wq, Wkv, Wo, bo)
    )
    Wk_g = Wkv[:D] * g[None, :]
    Wv_g = Wkv[D:] * g[None, :]
    wq2 = ((q @ Wq.T) @ Wk_g) * (D ** -0.5) * SQRT_C
    Mw2 = (Wo @ Wv_g) * SQRT_C

    consts = np.zeros((128, CONST_W), np.float32)
    for j in range(NH):
        consts[:, SSQS_OFF + j * 4 + j] = 1.0
    for b in range(B):
        for kc in range(KC):
            for j in range(NH):
                o = DOTSS_OFF + ((b * KC + kc) * NH + j) * 4
                consts[:, o + j] = wq2[b, kc * 128 : (kc + 1) * 128]
    for col in range(4):
        consts[0:4, ZSELS_OFF + col] = 1.0
        consts[32:36, ZSELS_OFF + col] = 1.0
    consts[:, ID_OFF : ID_OFF + 128] = np.eye(128, dtype=np.float32)
    for j in range(NH):
        consts[j, BSEL_OFF + j * 128 : BSEL_OFF + (j + 1) * 128] = 1.0
        consts[32 + j, BSEL_OFF + j * 128 : BSEL_OFF + (j + 1) * 128] = 1.0
    for kc in range(KC):
        consts[:, MWT_OFF + kc * C : MWT_OFF + (kc + 1) * C] = Mw2[
            :, kc * 128 : (kc + 1) * 128
        ].T
    bo2 = np.zeros((128, KC + 1), np.float32)
    bo2[:, :KC] = bo.reshape(KC, 128).T
    bo2[:, KC] = QFC
    return consts.astype(bf), bo2


_NC_CACHE = {}


def _get_nc():
    if "nc" not in _NC_CACHE:
        _NC_CACHE["nc"] = _build_nc()
    return _NC_CACHE["nc"]


def _run(q, c, g, Wq, Wkv, Wo, bo, trace=False):
    bf = mybir.dt.np(BF16)
    consts, bo2 = _host_consts(q, g, Wq, Wkv, Wo, bo)
    c_bf = np.asarray(c, np.float32).astype(bf)
    # [B,N,C,H,W] -> [core(H//HS), 128, B, H2, KC, NH, PIX]
    c_t = c_bf.reshape(B, H2, NH, KC, 128, H // HS, HS * W).transpose(
        5, 4, 0, 1, 3, 2, 6)
    in_maps = []
    for i in range(NCORES):
        shard = np.ascontiguousarray(c_t[i])
        in_maps.append({"c": shard, "consts": consts, "bo2": bo2})
    nc = _get_nc()
    res = run_bass_kernel_spmd(nc, in_maps, core_ids=list(range(NCORES)),
                               trace=trace)
    out = np.concatenate(
        [np.asarray(res.results[i]["out"]).astype(np.float32)
         for i in range(NCORES)],
        axis=2,
    )
    return out, res


def kernel(q, c, g, Wq, Wkv, Wo, bo):
    out, _ = _run(q, c, g, Wq, Wkv, Wo, bo, trace=False)
    return out


def kernel_traced(q, c, g, Wq, Wkv, Wo, bo):
    out, res = _run(q, c, g, Wq, Wkv, Wo, bo, trace=True)
    return out, res
